# revision 1
# baseline (speedup 1.0000x reference)
"""NT-Xent (SimCLR) contrastive loss on 8 Trainium2 NeuronCores — v2.5.

Collective-free data-parallel design. Each core owns 512 loss rows; the host
permutes the stacked embedding matrix per core so the own rows sit at rows
0..511 (i-half) and 4096..4607 (j-half) — a pure layout transform that makes
one SPMD program serve all cores (self col = m, positive col = 4096+m).

Per-core pipeline (all on-chip, no DRAM roundtrip for the transpose):
  - SWDGE cast-DMA loads E f32 -> SBUF bf16 in partition-MAJOR row layout
    (row = 1024*g + 128*c + p), 8 groups of 1024 rows.
  - Per group: DVE squares + reduce -> bf16 norms; ACT ln/exp -> 1/|e|;
    DVE tensor_scalar -> unit rows z (bf16); 16 PE transposes -> dedicated
    PSUM pool (bf16); ACT/DVE copy-cast PSUM -> SBUF fp8e4 z^T [128,2,8192].
  - PE fp8 DoubleRow matmuls (K=256 fused per instruction) compute the
    [512, 8192] logits block from z^T slices. Matmul bursts for column
    group g-1 are emitted AFTER group g's transposes, so the PE always has
    independent work while ACT/DVE drain logits tiles (decoupled PSUM pools
    keep transposes off the matmul/exp dependency chain).
  - exp+rowsum ([128,1024] tiles): ACT Exp(scale=2, accum_out) for most,
    bf16-Schraudolph fast-exp on DVE (mult-add -> int16 bits, bitcast bf16,
    reduce) for the rest, interleaved so both engines drain PSUM.
  - Self logit is 2|z|^2 ~= 2: subtract constant e^2 via the Ln bias.
  - Output per core: 512 per-row loss terms [128, 4]; host sums/4096.
"""

import sys

if "/opt/trn_rl_repo" not in sys.path:
    sys.path.insert(0, "/opt/trn_rl_repo")

import numpy as np

import concourse.bass as bass
import concourse.mybir as mybir
import concourse.tile as tile
from concourse import bass_utils
from concourse.masks import make_identity

N_CORES = 8
N = 4096
D = 256
R = 2 * N                 # 8192 stacked rows
NG = 8                    # row groups of 1024
OWN = N // N_CORES        # 512 loss rows per core
INV_T = 2.0
E2_SELF = float(np.float32(np.exp(np.float32(2.0))))

# bf16 Schraudolph fast exp(2*S): bits_i16 = round(S*A + B); bitcast bf16.
A_SCH = 369.3299304957    # 256 * log2(e)
B_SCH = 16251.0613        # calibrated for S ~ N(0, 1/16^2), mean-zero error

FP32 = mybir.dt.float32
BF16 = mybir.dt.bfloat16
FP8 = mybir.dt.float8e4
I16 = mybir.dt.int16

AF = mybir.ActivationFunctionType
ALU = mybir.AluOpType
PM = mybir.MatmulPerfMode

# exp engine per slot (slot = mb*8 + cc, cc = 1024-col chunk = source group):
# "A"=ACT exp, "D"=DVE Schraudolph. DVE takes a spread of mid-pipeline slots
# plus half of the final column so the tail drains on both engines.
_D_SLOTS = {8 + 2, 8 + 4, 24 + 3, 24 + 5, 8 + 7, 24 + 7}
EXP_MODE = {s: ("D" if s in _D_SLOTS else "A") for s in range(32)}
# copy engine per (group, khalf) index 0..15: alternate ACT/DVE
COPY_ENG = ["A", "D"] * 8


def _split_oversized_waits(nc, max_waits=1):
    """Walrus accepts at most one sync-wait per instruction; hoist extras
    onto preceding single-wait drains on the same engine (streams are FIFO
    per engine, so semantics are preserved)."""
    for bb in nc.main_func.blocks:
        new_list = []
        for ins in bb.instructions:
            si = ins.sync_info
            if si is not None and si.on_wait and len(si.on_wait) > max_waits:
                waits = list(si.on_wait)
                extra, keep = waits[:-max_waits], waits[-max_waits:]
                for gi, w in enumerate(extra):
                    d = mybir.InstDrain(name=f"{ins.name}-wsplit{gi}", engine=ins.engine)
                    d.sync_info = mybir.SyncInfo(on_wait=[w], on_update=[])
                    new_list.append(d)
                ins.sync_info = mybir.SyncInfo(on_wait=list(keep), on_update=list(si.on_update))
            new_list.append(ins)
        bb.instructions = new_list


def _build():
    nc = bass.Bass("TRN2", num_devices=N_CORES)
    e_full = nc.dram_tensor("e_full", [R, D], FP32, kind="ExternalInput")
    pp_out = nc.dram_tensor("pp_out", [128, 4], FP32, kind="ExternalOutput")

    # partition-major rows: row = 1024*g + 128*c + p
    ev = e_full.ap().rearrange("(g c p) d -> g p c d", p=128, c=8)

    with tile.TileContext(nc) as tc:
        with tc.tile_pool(name="persist", bufs=1) as persist, \
             tc.tile_pool(name="work", bufs=3) as work, \
             tc.tile_pool(name="sqp", bufs=2) as sqp, \
             tc.tile_pool(name="sm", bufs=4) as sm, \
             tc.tile_pool(name="etp", bufs=3) as etp, \
             tc.tile_pool(name="tpp", bufs=2, space="PSUM") as tpp, \
             tc.tile_pool(name="psum", bufs=2, space="PSUM") as psp:

            # prefetch all 8 group loads first (SWDGE f32->bf16 cast) so the
            # first transfer starts before the identity build occupies Pool
            ebs = []
            for g in range(NG):
                eb = work.tile([128, 8, D], BF16, tag=f"eb{g}", bufs=1,
                               name=f"eb{g}")
                nc.gpsimd.dma_start(eb, ev[g])
                ebs.append(eb)

            ident = persist.tile([128, 128], BF16)
            make_identity(nc, ident)
            neg_e2 = persist.tile([128, 1], FP32)
            nc.vector.memset(neg_e2, -E2_SELF)

            zt = persist.tile([128, 2, R], FP8)       # z^T, khalf-major
            z_i0 = persist.tile([128, 8, D], BF16)    # group 0 (own i rows)
            z_j0 = persist.tile([128, 8, D], BF16)    # group 4 (own j rows)
            rs = persist.tile([128, 32], FP32)        # exp row-sum partials
            pos2 = persist.tile([128, 4], FP32)
            ppsb = persist.tile([128, 4], FP32)

            def normalize(g):
                eb = ebs[g]
                sq = sqp.tile([128, 8, D], BF16, tag="sq")
                nc.vector.tensor_mul(sq, eb, eb)
                n2 = sm.tile([128, 8], BF16, tag="n2")
                with nc.allow_low_precision("bf16 row norms, 0.4% is fine here"):
                    nc.vector.tensor_reduce(n2, sq, axis=mybir.AxisListType.X,
                                            op=ALU.add)
                lg = sm.tile([128, 8], FP32, tag="lg")
                nc.scalar.activation(lg, n2, AF.Ln)
                inv = sm.tile([128, 8], FP32, tag="inv")
                nc.scalar.activation(inv, lg, AF.Exp, scale=-0.5)
                if g == 0:
                    z = z_i0
                elif g == 4:
                    z = z_j0
                else:
                    z = work.tile([128, 8, D], BF16, tag="z")
                for c in range(8):
                    nc.vector.tensor_scalar_mul(z[:, c, :], eb[:, c, :],
                                                inv[:, c:c + 1])
                return z

            def do_exp(slot, St):
                mode = EXP_MODE[slot]
                if mode == "A":
                    tr = etp.tile([128, 1024], BF16, tag="etr")
                    nc.scalar.activation(tr, St, AF.Exp, scale=INV_T,
                                         accum_out=rs[:, slot:slot + 1])
                else:
                    si = etp.tile([128, 1024], I16, tag="si")
                    nc.vector.tensor_scalar(si, St, A_SCH, B_SCH,
                                            op0=ALU.mult, op1=ALU.add)
                    nc.vector.tensor_reduce(rs[:, slot:slot + 1],
                                            si.bitcast(BF16),
                                            axis=mybir.AxisListType.X,
                                            op=ALU.add)

            def bursts(cc):
                """matmul + exp for cols [1024*cc, 1024*(cc+1))."""
                for mb in range(4):
                    slot = mb * 8 + cc
                    Sm = psp.tile([128, 1024], FP32, tag="S")
                    for j in range(2):
                        col = 1024 * cc + j * 512
                        nc.tensor.matmul(Sm[:, j * 512:(j + 1) * 512],
                                         zt[:, :, mb * 128:(mb + 1) * 128],
                                         zt[:, :, col:col + 512],
                                         start=True, stop=True,
                                         perf_mode=PM.DoubleRow)
                    do_exp(slot, Sm)
                    if cc == 7:
                        # final column: finish this row-block inline
                        rtot = sm.tile([128, 1], FP32, tag="rtot")
                        nc.vector.tensor_reduce(rtot, rs[:, mb * 8:(mb + 1) * 8],
                                                axis=mybir.AxisListType.X,
                                                op=ALU.add)
                        logden = sm.tile([128, 1], FP32, tag="logden")
                        nc.scalar.activation(logden, rtot, AF.Ln,
                                             bias=neg_e2[:, 0:1])
                        nc.vector.scalar_tensor_tensor(
                            out=ppsb[:, mb:mb + 1], in0=pos2[:, mb:mb + 1],
                            scalar=-INV_T, in1=logden, op0=ALU.mult, op1=ALU.add)

            ci = 0
            for g in range(NG):
                z = normalize(g)
                tp = tpp.tile([128, 2, 1024], BF16, tag="tp")
                for c in range(8):
                    for k2 in range(2):
                        nc.tensor.transpose(tp[:, k2, c * 128:(c + 1) * 128],
                                            z[:, c, k2 * 128:(k2 + 1) * 128],
                                            ident)
                for k2 in range(2):
                    src = tp[:, k2, :]
                    dst = zt[:, k2, 1024 * g:1024 * (g + 1)]
                    if COPY_ENG[ci] == "A":
                        nc.scalar.copy(dst, src)
                    else:
                        nc.vector.tensor_copy(dst, src)
                    ci += 1

                if g == 4:
                    # own-z dots for the positive pairs (z ready for g=0,4)
                    for c in range(4):
                        ptr = sqp.tile([128, D], BF16, tag="ptr")
                        nc.vector.tensor_mul(ptr, z_i0[:, c, :], z_j0[:, c, :])
                        nc.vector.tensor_reduce(pos2[:, c:c + 1], ptr,
                                                axis=mybir.AxisListType.X,
                                                op=ALU.add)

                if g >= 1:
                    bursts(g - 1)
            bursts(7)

            nc.sync.dma_start(pp_out.ap(), ppsb)

    _split_oversized_waits(nc)
    return nc


_NC_CACHE = None


def _get_nc():
    global _NC_CACHE
    if _NC_CACHE is None:
        _NC_CACHE = _build()
    return _NC_CACHE


def _make_in_maps(emb_i: np.ndarray, emb_j: np.ndarray):
    emb_i = np.ascontiguousarray(np.asarray(emb_i, dtype=np.float32))
    emb_j = np.ascontiguousarray(np.asarray(emb_j, dtype=np.float32))
    in_maps = []
    for c in range(N_CORES):
        lo, hi = c * OWN, (c + 1) * OWN
        ei = np.concatenate([emb_i[lo:hi], emb_i[:lo], emb_i[hi:]], axis=0)
        ej = np.concatenate([emb_j[lo:hi], emb_j[:lo], emb_j[hi:]], axis=0)
        in_maps.append({"e_full": np.ascontiguousarray(
            np.concatenate([ei, ej], axis=0))})
    return in_maps


def kernel(emb_i: np.ndarray, emb_j: np.ndarray) -> np.ndarray:
    nc = _get_nc()
    in_maps = _make_in_maps(emb_i, emb_j)
    res = bass_utils.run_bass_kernel_spmd(nc, in_maps, core_ids=list(range(N_CORES)))
    total = 0.0
    for c in range(N_CORES):
        total += res.results[c]["pp_out"].astype(np.float64).sum()
    return np.float32(total / N)

